# revision 24
# baseline (speedup 1.0000x reference)
"""Trainium2 Bass kernel for CustomTriangleMultiplicationOutgoing.

Reference computation (B=1, N=384, D=C=128):
    z_norm = LN(z) * g + b                        # over D
    left   = (z_norm@Wa + ba) * sigmoid(z_norm@Wga + bga) * mask
    right  = (z_norm@Wb + bb) * sigmoid(z_norm@Wgb + bgb) * mask
    z_out[i,j,c] = sum_k left[i,k,c] * right[j,k,c]
    z_out  = LN(z_out) * g_out + b_out            # over C
    out    = (z_out@Wo + bo) * sigmoid(z_norm@Wgo + bgo)

Key identity: row-wise LN commutes with the projection,
    LN(z) @ (g .* W) = (z * rstd) @ ((I - 11^T/D)(g .* W))
so the host passes zs = (z * rstd)^T in bf16 and centered/affine-folded
weights; the device does plain matmuls with NO LN work in phase 1.
Phase 3's LN over C uses the same centering on Wo; its mean comes from a
vector-accumulated column sum during phase 2, its mean-square from
partial-K matmuls against a ones vector fused into each phase-2 chunk.

Sharding: 1D over the first N (i) axis, 48 rows per core.  Pass A
computes only the gated right projection so the 4 c-chunked AllGathers
(bf16) start early; pass B (left + out-gate) and phase 2 (einsum, k on
partitions) hide under the collectives.  Pass A/B element-wise ops are
batched over pairs of row tiles to amortize per-instruction overhead.
"""

import numpy as np
import ml_dtypes

import concourse.bass as bass
import concourse.mybir as mybir
import concourse.tile as tile
from concourse import bacc
from concourse.masks import make_identity
from concourse.bass_utils import run_bass_kernel_spmd

F32 = mybir.dt.float32
BF16 = mybir.dt.bfloat16
EPS = 1e-5

B = 1
N_FULL = 384
D = 128
C = 128
W = 8  # cores
P = 128


def bcast_part(ap, parts):
    """Broadcast a [1, ...] AP across `parts` partitions (partition step 0)."""
    return bass.AP(tensor=ap.tensor, offset=ap.offset, ap=[[0, parts]] + ap.ap[1:])


def pair_ap(ap0, ap1):
    """Fuse two same-shape/stride APs into one with a [delta, 2] middle dim."""
    assert ap0.ap == ap1.ap and ap0.tensor is ap1.tensor
    delta = ap1.offset - ap0.offset
    return bass.AP(
        tensor=ap0.tensor, offset=ap0.offset,
        ap=[ap0.ap[0]] + [[delta, 2]] + ap0.ap[1:],
    )


def build_nc(n=N_FULL, with_bias=False, with_mask=False, nq=4):
    """Build the SPMD Bass program (same program on all 8 cores)."""
    assert n % P == 0 and n % W == 0
    SH = n // W          # rows of i per core
    KC = n // P          # 128-wide chunks of k
    NT = SH * n // P     # 128-row tiles per core (= SH*KC)
    CQ = C // nq         # c per AllGather chunk
    CQH = CQ // 2        # c per phase-2 half-load
    QP = CQ              # partitions per chunk in zt_all (c-sharded rows)

    nc = bacc.Bacc(None, num_devices=W)

    zs = nc.declare_dram_parameter("zs", [P, NT * P], BF16, isOutput=False)
    wbg = nc.declare_dram_parameter("wbg", [D, 2 * C], BF16, isOutput=False)
    wago = nc.declare_dram_parameter("wago", [D, 2 * C + D], BF16, isOutput=False)
    wo = nc.declare_dram_parameter("wo", [C, D], BF16, isOutput=False)
    if with_bias:
        bbg_p = nc.declare_dram_parameter("bbg", [1, 2 * C], F32, isOutput=False)
        bago_p = nc.declare_dram_parameter("bago", [1, 2 * C + D], F32, isOutput=False)
        bo_p = nc.declare_dram_parameter("bo", [1, D], F32, isOutput=False)
    if with_mask:
        mask_sh = nc.declare_dram_parameter("mask_sh", [P, NT], F32, isOutput=False)
    out_sh = nc.declare_dram_parameter("out_sh", [P, NT, D], F32, isOutput=True)

    # internal DRAM
    right_q = [nc.dram_tensor(f"right_{q}", [P, KC, CQ, SH], BF16) for q in range(nq)]
    gath_q = [
        nc.dram_tensor(f"gath_{q}", [W, P, KC, CQ, SH], BF16, addr_space="Shared")
        for q in range(nq)
    ]
    zout = nc.dram_tensor("zout", [C, SH * n], BF16)  # c-major einsum result

    with tile.TileContext(nc) as tc:
        with tc.tile_pool(name="singles", bufs=1) as singles:
            wbg_sb = singles.tile([D, 2 * C], BF16)
            nc.sync.dma_start(wbg_sb, wbg[:])
            wago_sb = singles.tile([D, 2 * C + D], BF16)
            nc.sync.dma_start(wago_sb, wago[:])
            wo_sb = singles.tile([C, D], BF16)
            nc.sync.dma_start(wo_sb, wo[:])
            ones_bf = singles.tile([P, 1], BF16)
            nc.vector.memset(ones_bf, 1.0)
            eps_sb = singles.tile([P, 1], F32)
            nc.vector.memset(eps_sb, EPS)
            ident = singles.tile([P, P], F32)
            make_identity(nc, ident)
            if with_bias:
                bbg_sb = singles.tile([P, 2 * C], F32)
                nc.sync.dma_start(bbg_sb, bcast_part(bbg_p[:], P))
                bago_sb = singles.tile([P, 2 * C + D], F32)
                nc.sync.dma_start(bago_sb, bcast_part(bago_p[:], P))
                bo_sb = singles.tile([P, D], F32)
                nc.sync.dma_start(bo_sb, bcast_part(bo_p[:], P))
            if with_mask:
                mask_sb = singles.tile([P, NT], F32)
                nc.sync.dma_start(mask_sb, mask_sh[:])

            # persistent stores
            gg_sb = singles.tile([P, NT, D], BF16)      # out-gate per row tile
            zt_all = singles.tile([C, NT * P], BF16)    # z_out, c on partitions
            S_acc = singles.tile([SH, n], F32)          # sum_c z_out
            nc.vector.memset(S_acc, 0.0)

            lpool = tc.alloc_tile_pool(name="lpool", bufs=1)
            L_sb = lpool.tile([P, KC, SH, C], BF16)     # left: [k, kc, i_loc, c]

            p1pool = tc.alloc_tile_pool(name="p1", bufs=1)
            zs_sb = p1pool.tile([P, NT * P], BF16)
            for ch in range(4):
                w4 = NT * P // 4
                nc.sync.dma_start(
                    zs_sb[:, ch * w4 : (ch + 1) * w4],
                    zs[:, ch * w4 : (ch + 1) * w4],
                )
            R_stage = p1pool.tile([P, KC, C, SH], BF16)  # right: [k, kc, c, j_loc]

            NPAIR = NT // 2

            # ---------------- pass A: right projection ----------------
            with (
                tc.tile_pool(name="pA_tmp", bufs=4) as tmpsA,
                tc.tile_pool(name="pA_psum", bufs=4, space="PSUM") as psumA,
            ):
                for pi in range(NPAIR):
                    t0, t1 = 2 * pi, 2 * pi + 1
                    ps = psumA.tile([P, 2, 2 * C], F32, tag="psA")
                    for j, t in enumerate((t0, t1)):
                        nc.tensor.matmul(
                            ps[:, j, :], lhsT=zs_sb[:, t * P : (t + 1) * P],
                            rhs=wbg_sb, start=True, stop=True,
                        )
                    if with_bias:
                        for j in range(2):
                            nc.vector.tensor_tensor(
                                ps[:, j, :], ps[:, j, :], bbg_sb,
                                mybir.AluOpType.add,
                            )
                    sgb = tmpsA.tile([P, 2, C], F32, tag="sgb")
                    nc.scalar.activation(
                        sgb, ps[:, :, C : 2 * C],
                        mybir.ActivationFunctionType.Sigmoid,
                    )
                    if with_mask:
                        for j, t in enumerate((t0, t1)):
                            nc.gpsimd.tensor_scalar_mul(
                                sgb[:, j, :], sgb[:, j, :], mask_sb[:, t : t + 1]
                            )
                    rout = pair_ap(
                        R_stage[:, t0 % KC, :, t0 // KC],
                        R_stage[:, t1 % KC, :, t1 // KC],
                    )
                    nc.vector.tensor_tensor(
                        rout, ps[:, :, 0:C], sgb, mybir.AluOpType.mult
                    )
                for q in range(nq):
                    nc.sync.dma_start(
                        right_q[q][:], R_stage[:, :, q * CQ : (q + 1) * CQ, :]
                    )

            # ---------------- AllGather right (c-chunked) ----------------
            for q in range(nq):
                nc.gpsimd.collective_compute(
                    "AllGather",
                    mybir.AluOpType.bypass,
                    replica_groups=[list(range(W))],
                    ins=[right_q[q][:]],
                    outs=[gath_q[q][:]],
                )

            # ---------------- pass B: left + out-gate ----------------
            with (
                tc.tile_pool(name="pB_tmp", bufs=4) as tmpsB,
                tc.tile_pool(name="pB_psum", bufs=4, space="PSUM") as psumB,
                tc.tile_pool(name="pG_psum", bufs=4, space="PSUM") as psumG,
            ):
                for pi in range(NPAIR):
                    t0, t1 = 2 * pi, 2 * pi + 1
                    ps = psumB.tile([P, 2, 2 * C], F32, tag="psB")
                    pg = psumG.tile([P, 2, D], F32, tag="psG")
                    for j, t in enumerate((t0, t1)):
                        nc.tensor.matmul(
                            ps[:, j, :], lhsT=zs_sb[:, t * P : (t + 1) * P],
                            rhs=wago_sb[:, 0 : 2 * C], start=True, stop=True,
                        )
                        nc.tensor.matmul(
                            pg[:, j, :], lhsT=zs_sb[:, t * P : (t + 1) * P],
                            rhs=wago_sb[:, 2 * C :], start=True, stop=True,
                        )
                    if with_bias:
                        for j in range(2):
                            nc.vector.tensor_tensor(
                                ps[:, j, :], ps[:, j, :], bago_sb[:, 0 : 2 * C],
                                mybir.AluOpType.add,
                            )
                            nc.vector.tensor_tensor(
                                pg[:, j, :], pg[:, j, :], bago_sb[:, 2 * C :],
                                mybir.AluOpType.add,
                            )
                    sga = tmpsB.tile([P, 2, C], F32, tag="sga")
                    nc.scalar.activation(
                        sga, ps[:, :, C : 2 * C],
                        mybir.ActivationFunctionType.Sigmoid,
                    )
                    nc.scalar.activation(
                        gg_sb[:, t0 : t0 + 2, :], pg,
                        mybir.ActivationFunctionType.Sigmoid,
                    )
                    if with_mask:
                        for j, t in enumerate((t0, t1)):
                            nc.gpsimd.tensor_scalar_mul(
                                sga[:, j, :], sga[:, j, :], mask_sb[:, t : t + 1]
                            )
                    lout = pair_ap(
                        L_sb[:, t0 % KC, t0 // KC, :],
                        L_sb[:, t1 % KC, t1 // KC, :],
                    )
                    nc.vector.tensor_tensor(
                        lout, ps[:, :, 0:C], sga, mybir.AluOpType.mult
                    )

            p1pool.release()  # zs, R_stage freed

            # ---------------- phase 2: einsum + fused z_out stats ----------------
            sqpool = tc.alloc_tile_pool(name="sq_psum", bufs=1, space="PSUM")
            # one column block per chunk-pair, self-contained matmul chains
            sq_ps = sqpool.tile([P, 2, NT], F32)  # sum_c z_out^2 per row tile
            with (
                tc.tile_pool(name="p2_r", bufs=2) as rpool,
                tc.tile_pool(name="p2_st", bufs=3) as stpool,
                tc.tile_pool(name="p2_sq", bufs=4) as sqtmp,
                tc.tile_pool(name="p2_psum", bufs=6, space="PSUM") as psum2,
            ):
                for q in range(nq):
                    for h in range(2):
                        Rh = rpool.tile([P, KC, W, CQH, SH], BF16, tag="rh")
                        for m in range(W):
                            nc.sync.dma_start(
                                Rh[:, :, m],
                                gath_q[q][m, :, :, h * CQH : (h + 1) * CQH, :],
                            )
                        for c4 in range(CQH // 4):
                            stb = stpool.tile([SH, 4, n], BF16, tag="stb")
                            for c_ in range(4):
                                cl = c4 * 4 + c_
                                c_glob = q * CQ + h * CQH + cl
                                ps = psum2.tile([SH, n], F32, tag="ps")
                                for kc in range(KC):
                                    nc.tensor.matmul(
                                        ps,
                                        lhsT=L_sb[:, kc, :, c_glob],
                                        rhs=Rh[:, kc, :, cl, :],
                                        start=(kc == 0),
                                        stop=(kc == KC - 1),
                                    )
                                if c_ % 2 == 0:
                                    nc.vector.tensor_copy(stb[:, c_, :], ps)
                                else:
                                    nc.scalar.copy(stb[:, c_, :], ps)
                                nc.vector.tensor_tensor(
                                    S_acc, S_acc, ps, mybir.AluOpType.add
                                )
                            c0 = q * CQ + h * CQH + c4 * 4
                            nc.sync.dma_start(
                                zout[c0 : c0 + 4].rearrange(
                                    "c (i j) -> i c j", i=SH
                                ),
                                stb,
                            )
                    # z_out c-rows for this chunk are final: fetch to SBUF
                    nc.sync.dma_start(
                        zt_all[q * QP : (q + 1) * QP, :],
                        zout[q * QP : (q + 1) * QP, :],
                    )
                    # fused partial mean-square over pairs of chunks
                    # (matmul base partition must be 0/32/64)
                    if q % 2 == 1:
                        lo = (q - 1) * QP
                        for t in range(NT):
                            zq = zt_all[lo : lo + 2 * QP, t * P : (t + 1) * P]
                            sqv = sqtmp.tile([P, P], BF16, tag="sqv")
                            sqs = sqv[lo : lo + 2 * QP, :]
                            nc.vector.tensor_tensor(
                                sqs, zq, zq, mybir.AluOpType.mult
                            )
                            nc.tensor.matmul(
                                sq_ps[:, q // 2, t : t + 1], lhsT=sqs,
                                rhs=ones_bf[lo : lo + 2 * QP, :],
                                start=True, stop=True,
                            )

            SQm = singles.tile([P, NT], F32)   # sum_c z_out^2, tile-major
            nc.vector.tensor_copy(SQm, sq_ps[:, 0, :])
            nc.vector.tensor_tensor(
                SQm, SQm, sq_ps[:, 1, :], mybir.AluOpType.add
            )
            sqpool.release()
            lpool.release()  # L_sb freed

            # ---------------- phase 3: LN(z_out) @ Wo * gate ----------------
            with (
                tc.tile_pool(name="p3_tmp", bufs=4) as t3,
                tc.tile_pool(name="p3_big", bufs=1) as big3,
                tc.tile_pool(name="p3_psum", bufs=4, space="PSUM") as psum3,
                tc.tile_pool(name="p3_tps", bufs=3, space="PSUM") as tpsum,
            ):
                # transpose S_acc [SH, n] -> Smat [P, NT] (tile-major stats)
                Smat = big3.tile([P, NT], F32)
                for jc in range(KC):
                    tp = tpsum.tile([P, SH], F32, tag="tp")
                    nc.tensor.transpose(
                        tp, S_acc[:, jc * P : (jc + 1) * P], ident[0:SH, 0:SH]
                    )
                    nc.vector.tensor_copy(
                        Smat[:].rearrange("p (i k) -> p k i", k=KC)[:, jc, :], tp
                    )
                mean = big3.tile([P, NT], F32)
                nc.vector.tensor_scalar_mul(mean, Smat, 1.0 / C)
                msq = big3.tile([P, NT], F32)
                nc.vector.tensor_scalar_mul(msq, SQm, 1.0 / C)
                var = big3.tile([P, NT], F32)
                nc.vector.tensor_tensor(var, mean, mean, mybir.AluOpType.mult)
                nc.vector.tensor_tensor(var, msq, var, mybir.AluOpType.subtract)
                rstd = big3.tile([P, NT], F32)
                nc.scalar.activation(
                    rstd, var, mybir.ActivationFunctionType.Sqrt, bias=eps_sb
                )
                nc.vector.reciprocal(rstd, rstd)

                ot_sb = big3.tile([P, NT, D], F32)
                for t in range(NT):
                    pr = psum3.tile([P, D], F32, tag="pr")
                    nc.tensor.matmul(
                        pr, lhsT=zt_all[:, t * P : (t + 1) * P], rhs=wo_sb,
                        start=True, stop=True,
                    )
                    if with_bias:
                        po = t3.tile([P, D], F32, tag="po")
                        nc.vector.tensor_scalar_mul(po, pr, rstd[:, t : t + 1])
                        nc.vector.tensor_tensor(po, po, bo_sb, mybir.AluOpType.add)
                        nc.vector.tensor_tensor(
                            ot_sb[:, t, :], po, gg_sb[:, t, :], mybir.AluOpType.mult
                        )
                    else:
                        nc.vector.scalar_tensor_tensor(
                            ot_sb[:, t, :], pr, rstd[:, t : t + 1],
                            gg_sb[:, t, :],
                            mybir.AluOpType.mult, mybir.AluOpType.mult,
                        )
                for ch in range(4):
                    t0 = NT // 4 * ch
                    t1 = NT // 4 * (ch + 1)
                    nc.sync.dma_start(
                        out_sh[:, t0:t1, :], ot_sb[:, t0:t1, :]
                    )

    nc.compile()
    return nc


_CACHE = {}


def _get_nc(n, with_bias, with_mask):
    key = (n, with_bias, with_mask)
    if key not in _CACHE:
        _CACHE[key] = build_nc(n=n, with_bias=with_bias, with_mask=with_mask)
    return _CACHE[key]


def prepare_host(z, mask, norm_g, norm_b, norm_out_g, norm_out_b,
                 Wa, ba, Wb, bb, Wga, bga, Wgb, bgb, Wo, bo, Wgo, bgo, n=N_FULL):
    """Fold norm affines + centering into weights; pre-normalize z rows."""
    f = np.asarray
    z = f(z, dtype=np.float32)
    mask = f(mask, dtype=np.float32)
    g = f(norm_g, np.float32)
    b = f(norm_b, np.float32)
    go = f(norm_out_g, np.float32)
    bo_n = f(norm_out_b, np.float32)

    # LN(z) @ W_aff + bias = (z*rstd) @ Wcen + (b @ W + bias),
    # Wcen = (I - J/D)(g .* W)
    def fold(Wm, bias):
        Wm = f(Wm, np.float32)
        Wg = g[:, None] * Wm
        Wcen = Wg - np.mean(Wg, axis=0, keepdims=True)
        return Wcen, f(bias, np.float32) + b @ Wm

    Wa_, ba_ = fold(Wa, ba)
    Wga_, bga_ = fold(Wga, bga)
    Wb_, bb_ = fold(Wb, bb)
    Wgb_, bgb_ = fold(Wgb, bgb)
    Wgo_, bgo_ = fold(Wgo, bgo)
    Wo32 = f(Wo, np.float32)
    Wog = go[:, None] * Wo32
    Wo_ = Wog - np.mean(Wog, axis=0, keepdims=True)
    bo_ = f(bo, np.float32) + bo_n @ Wo32

    bf = ml_dtypes.bfloat16
    wbg_h = np.concatenate([Wb_, Wgb_], axis=1).astype(bf)
    wago_h = np.concatenate([Wa_, Wga_, Wgo_], axis=1).astype(bf)
    wo_h = Wo_.astype(bf)
    bbg_h = np.concatenate([bb_, bgb_])[None, :].astype(np.float32)
    bago_h = np.concatenate([ba_, bga_, bgo_])[None, :].astype(np.float32)

    with_bias = bool(np.any(bbg_h) or np.any(bago_h) or np.any(bo_))
    with_mask = not bool(np.all(mask == 1.0))

    # host-side LN stats: rstd per row of z, folded into z itself
    zf = z[0].reshape(n * n, D)
    m = zf.mean(axis=1, keepdims=True)
    v = ((zf - m) ** 2).mean(axis=1, keepdims=True)
    r = 1.0 / np.sqrt(v + EPS)
    zsf = (zf * r).astype(np.float32)

    SH = n // W
    NT = SH * n // P
    in_maps = []
    for mi in range(W):
        rows = zsf[SH * n * mi : SH * n * (mi + 1)]  # [SH*n, D]
        zs_h = np.ascontiguousarray(rows.T).astype(bf)  # [D, SH*n]
        im = {
            "zs": zs_h,
            "wbg": wbg_h,
            "wago": wago_h,
            "wo": wo_h,
        }
        if with_bias:
            im["bbg"] = bbg_h
            im["bago"] = bago_h
            im["bo"] = bo_[None, :].astype(np.float32)
        if with_mask:
            msk = mask[0].reshape(n * n)[SH * n * mi : SH * n * (mi + 1)]
            im["mask_sh"] = np.ascontiguousarray(
                msk.reshape(NT, P).T
            ).astype(np.float32)
        in_maps.append(im)
    return in_maps, with_bias, with_mask


def unshard(results, n=N_FULL):
    """results: list of per-core out_sh arrays [P, NT, D] -> [1, n, n, D]."""
    SH = n // W
    NT = SH * n // P
    parts = []
    for mi in range(W):
        o = results[mi].reshape(P, NT, D)
        parts.append(o.transpose(1, 0, 2).reshape(SH, n, D))
    return np.concatenate(parts, axis=0)[None]


def kernel(**inputs):
    n = inputs["z"].shape[1]
    in_maps, with_bias, with_mask = prepare_host(**inputs, n=n)
    nc = _get_nc(n, with_bias, with_mask)
    res = run_bass_kernel_spmd(nc, in_maps, list(range(W)))
    out = unshard([res.results[m]["out_sh"] for m in range(W)], n=n)
    return out.astype(np.float32)


# revision 26
# speedup vs baseline: 1.0173x; 1.0173x over previous
"""Trainium2 Bass kernel for CustomTriangleMultiplicationOutgoing.

Reference computation (B=1, N=384, D=C=128):
    z_norm = LN(z) * g + b                        # over D
    left   = (z_norm@Wa + ba) * sigmoid(z_norm@Wga + bga) * mask
    right  = (z_norm@Wb + bb) * sigmoid(z_norm@Wgb + bgb) * mask
    z_out[i,j,c] = sum_k left[i,k,c] * right[j,k,c]
    z_out  = LN(z_out) * g_out + b_out            # over C
    out    = (z_out@Wo + bo) * sigmoid(z_norm@Wgo + bgo)

Key identity: row-wise LN commutes with the projection,
    LN(z) @ (g .* W) = (z * rstd) @ ((I - 11^T/D)(g .* W))
so the host passes zs = (z * rstd)^T in bf16 and centered/affine-folded
weights; the device does plain matmuls with NO LN work in phase 1.
Phase 3's LN over C uses the same centering on Wo; its mean comes from a
vector-accumulated column sum during phase 2, its mean-square from
partial-K matmuls against a ones vector fused into each phase-2 chunk.

Sharding: 1D over the first N (i) axis, 48 rows per core.  Pass A
computes only the gated right projection so the 4 c-chunked AllGathers
(bf16) start early; pass B (left + out-gate) and phase 2 (einsum, k on
partitions) hide under the collectives.  Pass A/B element-wise ops are
batched over pairs of row tiles to amortize per-instruction overhead.
"""

import numpy as np
import ml_dtypes

import concourse.bass as bass
import concourse.mybir as mybir
import concourse.tile as tile
from concourse import bacc
from concourse.masks import make_identity
from concourse.bass_utils import run_bass_kernel_spmd

F32 = mybir.dt.float32
BF16 = mybir.dt.bfloat16
EPS = 1e-5

B = 1
N_FULL = 384
D = 128
C = 128
W = 8  # cores
P = 128


def bcast_part(ap, parts):
    """Broadcast a [1, ...] AP across `parts` partitions (partition step 0)."""
    return bass.AP(tensor=ap.tensor, offset=ap.offset, ap=[[0, parts]] + ap.ap[1:])


def pair_ap(ap0, ap1):
    """Fuse two same-shape/stride APs into one with a [delta, 2] middle dim."""
    assert ap0.ap == ap1.ap and ap0.tensor is ap1.tensor
    delta = ap1.offset - ap0.offset
    return bass.AP(
        tensor=ap0.tensor, offset=ap0.offset,
        ap=[ap0.ap[0]] + [[delta, 2]] + ap0.ap[1:],
    )


def build_nc(n=N_FULL, with_bias=False, with_mask=False, nq=4):
    """Build the SPMD Bass program (same program on all 8 cores)."""
    assert n % P == 0 and n % W == 0
    SH = n // W          # rows of i per core
    KC = n // P          # 128-wide chunks of k
    NT = SH * n // P     # 128-row tiles per core (= SH*KC)
    CQ = C // nq         # c per AllGather chunk
    CQH = CQ // 2        # c per phase-2 half-load
    QP = CQ              # partitions per chunk in zt_all (c-sharded rows)

    nc = bacc.Bacc(None, num_devices=W)

    zs = nc.declare_dram_parameter("zs", [P, NT * P], BF16, isOutput=False)
    wbg = nc.declare_dram_parameter("wbg", [D, 2 * C], BF16, isOutput=False)
    wago = nc.declare_dram_parameter("wago", [D, 2 * C + D], BF16, isOutput=False)
    wo = nc.declare_dram_parameter("wo", [C, D], BF16, isOutput=False)
    if with_bias:
        bbg_p = nc.declare_dram_parameter("bbg", [1, 2 * C], F32, isOutput=False)
        bago_p = nc.declare_dram_parameter("bago", [1, 2 * C + D], F32, isOutput=False)
        bo_p = nc.declare_dram_parameter("bo", [1, D], F32, isOutput=False)
    if with_mask:
        mask_sh = nc.declare_dram_parameter("mask_sh", [P, NT], F32, isOutput=False)
    out_sh = nc.declare_dram_parameter("out_sh", [P, NT, D], F32, isOutput=True)

    # internal DRAM
    right_q = [nc.dram_tensor(f"right_{q}", [P, KC, CQ, SH], BF16) for q in range(nq)]
    gath_q = [
        nc.dram_tensor(f"gath_{q}", [W, P, KC, CQ, SH], BF16, addr_space="Shared")
        for q in range(nq)
    ]
    zout = nc.dram_tensor("zout", [C, SH * n], BF16)  # c-major einsum result

    with tile.TileContext(nc) as tc:
        with tc.tile_pool(name="singles", bufs=1) as singles:
            wbg_sb = singles.tile([D, 2 * C], BF16)
            nc.sync.dma_start(wbg_sb, wbg[:])
            wago_sb = singles.tile([D, 2 * C + D], BF16)
            nc.sync.dma_start(wago_sb, wago[:])
            wo_sb = singles.tile([C, D], BF16)
            nc.sync.dma_start(wo_sb, wo[:])
            ones_bf = singles.tile([P, 1], BF16)
            nc.vector.memset(ones_bf, 1.0)
            eps_sb = singles.tile([P, 1], F32)
            nc.vector.memset(eps_sb, EPS)
            ident = singles.tile([P, P], F32)
            make_identity(nc, ident)
            if with_bias:
                bbg_sb = singles.tile([P, 2 * C], F32)
                nc.sync.dma_start(bbg_sb, bcast_part(bbg_p[:], P))
                bago_sb = singles.tile([P, 2 * C + D], F32)
                nc.sync.dma_start(bago_sb, bcast_part(bago_p[:], P))
                bo_sb = singles.tile([P, D], F32)
                nc.sync.dma_start(bo_sb, bcast_part(bo_p[:], P))
            if with_mask:
                mask_sb = singles.tile([P, NT], F32)
                nc.sync.dma_start(mask_sb, mask_sh[:])

            # persistent stores
            gg_sb = singles.tile([P, NT, D], BF16)      # out-gate per row tile
            zt_all = singles.tile([C, NT * P], BF16)    # z_out, c on partitions
            S_acc = singles.tile([SH, n], F32)          # sum_c z_out
            nc.vector.memset(S_acc, 0.0)

            lpool = tc.alloc_tile_pool(name="lpool", bufs=1)
            L_sb = lpool.tile([P, KC, SH, C], BF16)     # left: [k, kc, i_loc, c]

            p1pool = tc.alloc_tile_pool(name="p1", bufs=1)
            zs_sb = p1pool.tile([P, NT * P], BF16)
            for ch in range(8):
                w8 = NT * P // 8
                nc.sync.dma_start(
                    zs_sb[:, ch * w8 : (ch + 1) * w8],
                    zs[:, ch * w8 : (ch + 1) * w8],
                )
            R_stage = p1pool.tile([P, KC, C, SH], BF16)  # right: [k, kc, c, j_loc]

            NPAIR = NT // 2

            # ---------------- pass A: right projection ----------------
            with (
                tc.tile_pool(name="pA_tmp", bufs=4) as tmpsA,
                tc.tile_pool(name="pA_psum", bufs=4, space="PSUM") as psumA,
            ):
                for pi in range(NPAIR):
                    t0, t1 = 2 * pi, 2 * pi + 1
                    ps = psumA.tile([P, 2, 2 * C], F32, tag="psA")
                    for j, t in enumerate((t0, t1)):
                        nc.tensor.matmul(
                            ps[:, j, :], lhsT=zs_sb[:, t * P : (t + 1) * P],
                            rhs=wbg_sb, start=True, stop=True,
                        )
                    if with_bias:
                        for j in range(2):
                            nc.vector.tensor_tensor(
                                ps[:, j, :], ps[:, j, :], bbg_sb,
                                mybir.AluOpType.add,
                            )
                    sgb = tmpsA.tile([P, 2, C], BF16, tag="sgb")
                    nc.scalar.activation(
                        sgb, ps[:, :, C : 2 * C],
                        mybir.ActivationFunctionType.Sigmoid,
                    )
                    bcp = tmpsA.tile([P, 2, C], BF16, tag="bcp")
                    nc.scalar.copy(bcp, ps[:, :, 0:C])
                    if with_mask:
                        for j, t in enumerate((t0, t1)):
                            nc.gpsimd.tensor_scalar_mul(
                                sgb[:, j, :], sgb[:, j, :], mask_sb[:, t : t + 1]
                            )
                    rout = pair_ap(
                        R_stage[:, t0 % KC, :, t0 // KC],
                        R_stage[:, t1 % KC, :, t1 // KC],
                    )
                    nc.vector.tensor_tensor(
                        rout, bcp, sgb, mybir.AluOpType.mult
                    )
                for q in range(nq):
                    nc.sync.dma_start(
                        right_q[q][:], R_stage[:, :, q * CQ : (q + 1) * CQ, :]
                    )

            # ---------------- AllGather right (c-chunked) ----------------
            for q in range(nq):
                nc.gpsimd.collective_compute(
                    "AllGather",
                    mybir.AluOpType.bypass,
                    replica_groups=[list(range(W))],
                    ins=[right_q[q][:]],
                    outs=[gath_q[q][:]],
                )

            # ---------------- pass B: left + out-gate ----------------
            with (
                tc.tile_pool(name="pB_tmp", bufs=4) as tmpsB,
                tc.tile_pool(name="pB_psum", bufs=4, space="PSUM") as psumB,
                tc.tile_pool(name="pG_psum", bufs=4, space="PSUM") as psumG,
            ):
                for pi in range(NPAIR):
                    t0, t1 = 2 * pi, 2 * pi + 1
                    ps = psumB.tile([P, 2, 2 * C], F32, tag="psB")
                    pg = psumG.tile([P, 2, D], F32, tag="psG")
                    for j, t in enumerate((t0, t1)):
                        nc.tensor.matmul(
                            ps[:, j, :], lhsT=zs_sb[:, t * P : (t + 1) * P],
                            rhs=wago_sb[:, 0 : 2 * C], start=True, stop=True,
                        )
                        nc.tensor.matmul(
                            pg[:, j, :], lhsT=zs_sb[:, t * P : (t + 1) * P],
                            rhs=wago_sb[:, 2 * C :], start=True, stop=True,
                        )
                    if with_bias:
                        for j in range(2):
                            nc.vector.tensor_tensor(
                                ps[:, j, :], ps[:, j, :], bago_sb[:, 0 : 2 * C],
                                mybir.AluOpType.add,
                            )
                            nc.vector.tensor_tensor(
                                pg[:, j, :], pg[:, j, :], bago_sb[:, 2 * C :],
                                mybir.AluOpType.add,
                            )
                    sga = tmpsB.tile([P, 2, C], F32, tag="sga")
                    nc.scalar.activation(
                        sga, ps[:, :, C : 2 * C],
                        mybir.ActivationFunctionType.Sigmoid,
                    )
                    nc.scalar.activation(
                        gg_sb[:, t0 : t0 + 2, :], pg,
                        mybir.ActivationFunctionType.Sigmoid,
                    )
                    if with_mask:
                        for j, t in enumerate((t0, t1)):
                            nc.gpsimd.tensor_scalar_mul(
                                sga[:, j, :], sga[:, j, :], mask_sb[:, t : t + 1]
                            )
                    lout = pair_ap(
                        L_sb[:, t0 % KC, t0 // KC, :],
                        L_sb[:, t1 % KC, t1 // KC, :],
                    )
                    nc.vector.tensor_tensor(
                        lout, ps[:, :, 0:C], sga, mybir.AluOpType.mult
                    )

            p1pool.release()  # zs, R_stage freed

            # ---------------- phase 2: einsum + fused z_out stats ----------------
            sqpool = tc.alloc_tile_pool(name="sq_psum", bufs=1, space="PSUM")
            # one column block per chunk-pair, self-contained matmul chains
            sq_ps = sqpool.tile([P, 2, NT], F32)  # sum_c z_out^2 per row tile
            with (
                tc.tile_pool(name="p2_r", bufs=2) as rpool,
                tc.tile_pool(name="p2_st", bufs=3) as stpool,
                tc.tile_pool(name="p2_sq", bufs=4) as sqtmp,
                tc.tile_pool(name="p2_psum", bufs=6, space="PSUM") as psum2,
            ):
                for q in range(nq):
                    for h in range(2):
                        Rh = rpool.tile([P, KC, W, CQH, SH], BF16, tag="rh")
                        for m in range(W):
                            nc.sync.dma_start(
                                Rh[:, :, m],
                                gath_q[q][m, :, :, h * CQH : (h + 1) * CQH, :],
                            )
                        for c4 in range(CQH // 4):
                            stb = stpool.tile([SH, 4, n], BF16, tag="stb")
                            for c_ in range(4):
                                cl = c4 * 4 + c_
                                c_glob = q * CQ + h * CQH + cl
                                ps = psum2.tile([SH, n], F32, tag="ps")
                                for kc in range(KC):
                                    nc.tensor.matmul(
                                        ps,
                                        lhsT=L_sb[:, kc, :, c_glob],
                                        rhs=Rh[:, kc, :, cl, :],
                                        start=(kc == 0),
                                        stop=(kc == KC - 1),
                                    )
                                if c_ % 2 == 0:
                                    nc.vector.tensor_copy(stb[:, c_, :], ps)
                                else:
                                    nc.scalar.copy(stb[:, c_, :], ps)
                                nc.vector.tensor_tensor(
                                    S_acc, S_acc, ps, mybir.AluOpType.add
                                )
                            c0 = q * CQ + h * CQH + c4 * 4
                            nc.sync.dma_start(
                                zout[c0 : c0 + 4].rearrange(
                                    "c (i j) -> i c j", i=SH
                                ),
                                stb,
                            )
                    # z_out c-rows for this chunk are final: fetch to SBUF
                    nc.sync.dma_start(
                        zt_all[q * QP : (q + 1) * QP, :],
                        zout[q * QP : (q + 1) * QP, :],
                    )
                    # fused partial mean-square over pairs of chunks
                    # (matmul base partition must be 0/32/64)
                    if q % 2 == 1:
                        lo = (q - 1) * QP
                        for t in range(NT):
                            zq = zt_all[lo : lo + 2 * QP, t * P : (t + 1) * P]
                            sqv = sqtmp.tile([P, P], BF16, tag="sqv")
                            sqs = sqv[lo : lo + 2 * QP, :]
                            nc.vector.tensor_tensor(
                                sqs, zq, zq, mybir.AluOpType.mult
                            )
                            nc.tensor.matmul(
                                sq_ps[:, q // 2, t : t + 1], lhsT=sqs,
                                rhs=ones_bf[lo : lo + 2 * QP, :],
                                start=True, stop=True,
                            )

            SQm = singles.tile([P, NT], F32)   # sum_c z_out^2, tile-major
            nc.vector.tensor_copy(SQm, sq_ps[:, 0, :])
            nc.vector.tensor_tensor(
                SQm, SQm, sq_ps[:, 1, :], mybir.AluOpType.add
            )
            sqpool.release()
            lpool.release()  # L_sb freed

            # ---------------- phase 3: LN(z_out) @ Wo * gate ----------------
            with (
                tc.tile_pool(name="p3_tmp", bufs=4) as t3,
                tc.tile_pool(name="p3_big", bufs=1) as big3,
                tc.tile_pool(name="p3_psum", bufs=4, space="PSUM") as psum3,
                tc.tile_pool(name="p3_tps", bufs=3, space="PSUM") as tpsum,
            ):
                # transpose S_acc [SH, n] -> Smat [P, NT] (tile-major stats)
                Smat = big3.tile([P, NT], F32)
                for jc in range(KC):
                    tp = tpsum.tile([P, SH], F32, tag="tp")
                    nc.tensor.transpose(
                        tp, S_acc[:, jc * P : (jc + 1) * P], ident[0:SH, 0:SH]
                    )
                    nc.vector.tensor_copy(
                        Smat[:].rearrange("p (i k) -> p k i", k=KC)[:, jc, :], tp
                    )
                mean = big3.tile([P, NT], F32)
                nc.vector.tensor_scalar_mul(mean, Smat, 1.0 / C)
                msq = big3.tile([P, NT], F32)
                nc.vector.tensor_scalar_mul(msq, SQm, 1.0 / C)
                var = big3.tile([P, NT], F32)
                nc.vector.tensor_tensor(var, mean, mean, mybir.AluOpType.mult)
                nc.vector.tensor_tensor(var, msq, var, mybir.AluOpType.subtract)
                rstd = big3.tile([P, NT], F32)
                nc.scalar.activation(
                    rstd, var, mybir.ActivationFunctionType.Sqrt, bias=eps_sb
                )
                nc.vector.reciprocal(rstd, rstd)

                ot_sb = big3.tile([P, NT, D], F32)
                for t in range(NT):
                    pr = psum3.tile([P, D], F32, tag="pr")
                    nc.tensor.matmul(
                        pr, lhsT=zt_all[:, t * P : (t + 1) * P], rhs=wo_sb,
                        start=True, stop=True,
                    )
                    if with_bias:
                        po = t3.tile([P, D], F32, tag="po")
                        nc.vector.tensor_scalar_mul(po, pr, rstd[:, t : t + 1])
                        nc.vector.tensor_tensor(po, po, bo_sb, mybir.AluOpType.add)
                        nc.vector.tensor_tensor(
                            ot_sb[:, t, :], po, gg_sb[:, t, :], mybir.AluOpType.mult
                        )
                    else:
                        nc.vector.scalar_tensor_tensor(
                            ot_sb[:, t, :], pr, rstd[:, t : t + 1],
                            gg_sb[:, t, :],
                            mybir.AluOpType.mult, mybir.AluOpType.mult,
                        )
                for ch in range(4):
                    t0 = NT // 4 * ch
                    t1 = NT // 4 * (ch + 1)
                    nc.sync.dma_start(
                        out_sh[:, t0:t1, :], ot_sb[:, t0:t1, :]
                    )

    nc.compile()
    return nc


_CACHE = {}


def _get_nc(n, with_bias, with_mask):
    key = (n, with_bias, with_mask)
    if key not in _CACHE:
        _CACHE[key] = build_nc(n=n, with_bias=with_bias, with_mask=with_mask)
    return _CACHE[key]


def prepare_host(z, mask, norm_g, norm_b, norm_out_g, norm_out_b,
                 Wa, ba, Wb, bb, Wga, bga, Wgb, bgb, Wo, bo, Wgo, bgo, n=N_FULL):
    """Fold norm affines + centering into weights; pre-normalize z rows."""
    f = np.asarray
    z = f(z, dtype=np.float32)
    mask = f(mask, dtype=np.float32)
    g = f(norm_g, np.float32)
    b = f(norm_b, np.float32)
    go = f(norm_out_g, np.float32)
    bo_n = f(norm_out_b, np.float32)

    # LN(z) @ W_aff + bias = (z*rstd) @ Wcen + (b @ W + bias),
    # Wcen = (I - J/D)(g .* W)
    def fold(Wm, bias):
        Wm = f(Wm, np.float32)
        Wg = g[:, None] * Wm
        Wcen = Wg - np.mean(Wg, axis=0, keepdims=True)
        return Wcen, f(bias, np.float32) + b @ Wm

    Wa_, ba_ = fold(Wa, ba)
    Wga_, bga_ = fold(Wga, bga)
    Wb_, bb_ = fold(Wb, bb)
    Wgb_, bgb_ = fold(Wgb, bgb)
    Wgo_, bgo_ = fold(Wgo, bgo)
    Wo32 = f(Wo, np.float32)
    Wog = go[:, None] * Wo32
    Wo_ = Wog - np.mean(Wog, axis=0, keepdims=True)
    bo_ = f(bo, np.float32) + bo_n @ Wo32

    bf = ml_dtypes.bfloat16
    wbg_h = np.concatenate([Wb_, Wgb_], axis=1).astype(bf)
    wago_h = np.concatenate([Wa_, Wga_, Wgo_], axis=1).astype(bf)
    wo_h = Wo_.astype(bf)
    bbg_h = np.concatenate([bb_, bgb_])[None, :].astype(np.float32)
    bago_h = np.concatenate([ba_, bga_, bgo_])[None, :].astype(np.float32)

    with_bias = bool(np.any(bbg_h) or np.any(bago_h) or np.any(bo_))
    with_mask = not bool(np.all(mask == 1.0))

    # host-side LN stats: rstd per row of z, folded into z itself
    zf = z[0].reshape(n * n, D)
    m = zf.mean(axis=1, keepdims=True)
    v = ((zf - m) ** 2).mean(axis=1, keepdims=True)
    r = 1.0 / np.sqrt(v + EPS)
    zsf = (zf * r).astype(np.float32)

    SH = n // W
    NT = SH * n // P
    in_maps = []
    for mi in range(W):
        rows = zsf[SH * n * mi : SH * n * (mi + 1)]  # [SH*n, D]
        zs_h = np.ascontiguousarray(rows.T).astype(bf)  # [D, SH*n]
        im = {
            "zs": zs_h,
            "wbg": wbg_h,
            "wago": wago_h,
            "wo": wo_h,
        }
        if with_bias:
            im["bbg"] = bbg_h
            im["bago"] = bago_h
            im["bo"] = bo_[None, :].astype(np.float32)
        if with_mask:
            msk = mask[0].reshape(n * n)[SH * n * mi : SH * n * (mi + 1)]
            im["mask_sh"] = np.ascontiguousarray(
                msk.reshape(NT, P).T
            ).astype(np.float32)
        in_maps.append(im)
    return in_maps, with_bias, with_mask


def unshard(results, n=N_FULL):
    """results: list of per-core out_sh arrays [P, NT, D] -> [1, n, n, D]."""
    SH = n // W
    NT = SH * n // P
    parts = []
    for mi in range(W):
        o = results[mi].reshape(P, NT, D)
        parts.append(o.transpose(1, 0, 2).reshape(SH, n, D))
    return np.concatenate(parts, axis=0)[None]


def kernel(**inputs):
    n = inputs["z"].shape[1]
    in_maps, with_bias, with_mask = prepare_host(**inputs, n=n)
    nc = _get_nc(n, with_bias, with_mask)
    res = run_bass_kernel_spmd(nc, in_maps, list(range(W)))
    out = unshard([res.results[m]["out_sh"] for m in range(W)], n=n)
    return out.astype(np.float32)
